# revision 1
# baseline (speedup 1.0000x reference)
"""AttentionBlock kernel for 8x Trainium2 NeuronCores.

Strategy: data-parallel over batch (B=8 -> 1 batch element per core).
Per core everything is computed in a channel-major ("transposed") layout so
that no on-chip transposes are needed anywhere:

  x slice in HBM is [C=512, N=1024]  (exactly xs^T)
  qk projection  -> qkT[och, tok]   (lhsT = Wp tile, rhs = x tile)
  v  projection  -> v[tok, och]     (lhsT = x tile,  rhs = Wp v-columns)
  S^T[j, i]      -> lhsT = kT jtile, rhs = qT ichunk        (d contraction)
  E = exp(scale * S^T)  on ScalarE (PSUM -> SBUF)
  O^T[d, i]      -> sum_j  lhsT = v[jtile, head cols], rhs = E[jtile]
  colsums        -> a banded-ones lhsT accumulates (head, chunk) colsums
                    into rows of two [4, 512] PSUM tiles (heads 0-1 / 2-3);
                    each batched reciprocal + K=4 selector broadcast matmul
                    runs while later heads still compute, so softmax
                    normalization never stalls the in-order PE stream.
  out^T[c, tok]  -> lhsT = Wo tile, rhs = O^T  (+ bres + x residual, one DVE op)

The v bias is folded through the output projection (sum_j P = 1), as
bres = bo + Wo^T bv, computed on device with 16 tiny matmuls pinned into
the PE stream right where the last reciprocal runs.

Matmul operands are bf16 (half the HBM traffic, fast weight load, and the
matmul stream runs at the 216 ns/512-row warm cadence); the softmax
normalization chain stays f32/f32r and the residual add reads the original
fp32 x (loaded late, off the startup critical path).  PSUM working tiles
span two banks [128, 1024] so ScalarE/VectorE run one op per tile pair.
"""

import sys

sys.path.insert(0, "/opt/trn_rl_repo")

import numpy as np

import concourse.bass as bass
import concourse.tile as tile
import concourse.mybir as mybir
from concourse.bass_utils import run_bass_kernel_spmd
from bass_rust import add_dep_helper

B, C, HW = 8, 512, 1024
NH, DK = 4, 128
OCH = NH * DK * 3  # 1536
SCALE = float(DK) ** -0.5
F32 = mybir.dt.float32
F32R = mybir.dt.float32r
BF16DT = mybir.dt.bfloat16

MM_DTYPE = "bf16"  # "bf16" | "f32r"
BF16 = MM_DTYPE == "bf16"
MMDT = BF16DT if BF16 else F32R

# ---------------------------------------------------------------------------
# Walrus in this container supports only ONE embedded sync-wait per
# instruction ("Too many sync wait commands" otherwise).  Tile emits
# multi-wait instructions, so rewrite: each instruction keeps its last wait
# and gets N-1 single-wait NoOps inserted right before it on the same engine.
# ---------------------------------------------------------------------------
_wsplit_counter = [0]


def _split_multi_waits(nc):
    for fn in nc.m.functions:
        for blk in fn.blocks:
            insts = blk.instructions
            if not insts:
                continue
            new = []
            changed = False
            for inst in insts:
                si = inst.sync_info
                waits = list(si.on_wait) if si is not None and si.on_wait else []
                if len(waits) > 1:
                    changed = True
                    for w in waits[:-1]:
                        _wsplit_counter[0] += 1
                        nop = mybir.InstNoOp(
                            name=f"WSPLIT-{_wsplit_counter[0]}",
                            ins=[],
                            outs=[],
                            engine=inst.engine,
                        )
                        nop.sync_info = mybir.SyncInfo(on_wait=[w], on_update=[])
                        nc.register_instruction(nop, overwrite=True)
                        new.append(nop)
                    inst.sync_info = mybir.SyncInfo(
                        on_wait=[waits[-1]], on_update=list(si.on_update or [])
                    )
                new.append(inst)
            if changed:
                blk.instructions = new


def build_attention_nc():
    nc = bass.Bass("TRN2")
    xb = nc.dram_tensor("xb", [C, HW], MMDT, kind="ExternalInput")
    xf = nc.dram_tensor("xf", [C, HW], F32, kind="ExternalInput")
    wp = nc.dram_tensor("wp", [C, OCH], MMDT, kind="ExternalInput")
    bp = nc.dram_tensor("bp", [OCH, 1], MMDT, kind="ExternalInput")
    wo = nc.dram_tensor("wo", [NH * DK, C], MMDT, kind="ExternalInput")
    bo = nc.dram_tensor("bo", [C, 1], F32, kind="ExternalInput")
    bpf = nc.dram_tensor("bpf", [OCH, 1], F32, kind="ExternalInput")
    # colsum row-selector band: tband[:, c] = 1 iff c == 3, so the slice
    # tband[:, 3-q : 7-q] is a [128, 4] matrix whose only ones-column is q
    tband = nc.dram_tensor("tband", [128, 7], MMDT, kind="ExternalInput")
    # broadcast row-selector band: uband[k, (3-q)*128 : (4-q)*128] == (k == q)
    uband = nc.dram_tensor("uband", [4, 7 * 128], F32R, kind="ExternalInput")
    out = nc.dram_tensor("out", [C, HW], F32, kind="ExternalOutput")

    xb, xf, wp, bp, wo, bo, bpf, tband, uband, out = (
        t.ap() for t in (xb, xf, wp, bp, wo, bo, bpf, tband, uband, out)
    )

    def r(ap):
        return ap

    EXP = mybir.ActivationFunctionType.Exp
    ADD = mybir.AluOpType.add
    MUL = mybir.AluOpType.mult
    IC = [slice(0, 512), slice(512, 1024)]

    with tile.TileContext(nc) as tc:
        with (
            tc.tile_pool(name="persist", bufs=1) as persist,
            tc.tile_pool(name="epool", bufs=24) as epool,
            tc.tile_pool(name="outp", bufs=4) as outp,
            tc.tile_pool(name="psA", bufs=3, space="PSUM") as ps_main,
            tc.tile_pool(name="psC", bufs=2, space="PSUM") as ps_cs,
        ):
            # ---- persistent SBUF tensors -------------------------------
            xb_sb = [persist.tile([128, HW], MMDT, tag=f"xb{i}", name=f"xb{i}") for i in range(4)]
            xf_sb = [persist.tile([128, HW], F32, tag=f"xf{i}", name=f"xf{i}") for i in range(4)]
            wp_sb = [persist.tile([128, OCH], MMDT, tag=f"wp{i}", name=f"wp{i}") for i in range(4)]
            wo_sb = [persist.tile([128, C], MMDT, tag=f"wo{i}", name=f"wo{i}") for i in range(4)]
            # q^T / k^T per head: index h*2 + (0=q, 1=k)
            qk_sb = [persist.tile([128, HW], MMDT, tag=f"qk{i}", name=f"qk{i}") for i in range(8)]
            # v in [tok, (h, d)] layout, 8 token tiles
            v_sb = [persist.tile([128, NH * DK], MMDT, tag=f"v{i}", name=f"v{i}") for i in range(8)]
            # attention output O^T (unnormalized, then normalized in place)
            oT_sb = [persist.tile([128, HW], MMDT, tag=f"oT{i}", name=f"oT{i}") for i in range(4)]
            bqk_sb = [persist.tile([128, 1], F32, tag=f"bqk{i}", name=f"bqk{i}") for i in range(8)]
            bv_sb = [persist.tile([128, 1], MMDT, tag=f"bv{i}", name=f"bv{i}") for i in range(4)]
            bo_sb = [persist.tile([128, 1], F32, tag=f"bo{i}", name=f"bo{i}") for i in range(4)]
            bres_sb = [persist.tile([128, 1], F32, tag=f"bres{i}", name=f"bres{i}") for i in range(4)]
            t_sb = persist.tile([128, 7], MMDT, tag="t_sb", name="t_sb")
            u_sb = persist.tile([4, 7 * 128], F32R, tag="u_sb", name="u_sb")
            csr01 = persist.tile([4, 512], F32R, tag="csr01", name="csr01")
            csr23 = persist.tile([4, 512], F32R, tag="csr23", name="csr23")

            # ---- loads -------------------------------------------------
            # bf16 x first on the sync HWDGE queue (startup critical path);
            # wp per (row-tile, head col-group) on gpsimd; fp32 x afterwards
            # on gpsimd (only needed by the residual add at the very end);
            # weights/consts on the scalar queue.
            for i in range(4):
                nc.sync.dma_start(out=xb_sb[i], in_=xb[i * 128 : (i + 1) * 128, :])
            for h in range(NH):
                for kc in range(4):
                    c0 = h * 384
                    nc.gpsimd.dma_start(
                        out=wp_sb[kc][:, c0 : c0 + 384],
                        in_=wp[kc * 128 : (kc + 1) * 128, c0 : c0 + 384],
                    )
            for i in range(4):
                nc.gpsimd.dma_start(out=xf_sb[i], in_=xf[i * 128 : (i + 1) * 128, :])
            for h in range(NH):
                for qk in range(2):
                    o0 = h * 384 + qk * 128
                    nc.scalar.dma_start(
                        out=bqk_sb[h * 2 + qk], in_=bpf[o0 : o0 + 128, 0:1]
                    )
                o0 = h * 384 + 256
                nc.scalar.dma_start(out=bv_sb[h], in_=bp[o0 : o0 + 128, 0:1])
            nc.scalar.dma_start(out=t_sb, in_=tband[:, :])
            nc.scalar.dma_start(out=u_sb, in_=uband[:, :])
            for i in range(4):
                nc.scalar.dma_start(out=bo_sb[i], in_=bo[i * 128 : (i + 1) * 128, 0:1])
                nc.scalar.dma_start(out=wo_sb[i], in_=wo[i * 128 : (i + 1) * 128, :])

            # ---- phase A: q/k projection, [och, tok] layout ------------
            for h in range(NH):
                pss = [
                    ps_main.tile([128, HW], F32, tag="psA", name="psA")
                    for _ in range(2)
                ]
                for kc in range(4):
                    for qk in range(2):
                        o0 = h * 384 + qk * 128
                        for ic in range(2):
                            nc.tensor.matmul(
                                pss[qk][:, IC[ic]],
                                r(wp_sb[kc][:, o0 : o0 + 128]),
                                r(xb_sb[kc][:, IC[ic]]),
                                start=(kc == 0),
                                stop=(kc == 3),
                            )
                for qk in range(2):
                    nc.vector.tensor_scalar_add(
                        out=qk_sb[h * 2 + qk][:],
                        in0=pss[qk][:],
                        scalar1=bqk_sb[h * 2 + qk][:],
                    )

            # ---- phase B: v projection, [tok, (h, d)] layout -----------
            wp_v = [
                wp_sb[kc][:].rearrange("p (h t) -> p h t", h=4)[:, :, 256:384]
                for kc in range(4)
            ]
            for jp in range(4):
                ps = ps_main.tile([128, HW], F32, tag="psA", name="psA")
                for jh in range(2):
                    jt = jp * 2 + jh
                    for kc in range(4):
                        nc.tensor.matmul(
                            ps[:, IC[jh]],
                            r(xb_sb[kc][:, jt * 128 : (jt + 1) * 128]),
                            r(wp_v[kc]),
                            start=(kc == 0),
                            stop=(kc == 3),
                        )
                nc.vector.tensor_copy(out=v_sb[jp * 2][:], in_=ps[:, 0:512])
                nc.vector.tensor_copy(out=v_sb[jp * 2 + 1][:], in_=ps[:, 512:1024])

            # ---- phase C: attention per head ---------------------------
            psc = [
                ps_cs.tile([4, 512], F32, tag="psC", name="psC") for _ in range(2)
            ]
            csr = [csr01, csr23]
            last_cs = [None, None]

            def attention_head(h, defer_copy=False):
                half = h // 2
                q_loc = (h % 2) * 2  # row base within this half's psc tile
                qT = qk_sb[h * 2 + 0]
                kT = qk_sb[h * 2 + 1]
                E = []
                for jt in range(8):
                    ps = ps_main.tile([128, HW], F32, tag="psA", name="psA")
                    for ic in range(2):
                        nc.tensor.matmul(
                            ps[:, IC[ic]],
                            r(kT[:, jt * 128 : (jt + 1) * 128]),
                            r(qT[:, IC[ic]]),
                        )
                    e = epool.tile([128, HW], MMDT, tag="E", name="E")
                    nc.scalar.activation(out=e[:], in_=ps[:], func=EXP, scale=SCALE)
                    E.append(e)
                pso = ps_main.tile([128, HW], F32, tag="psA", name="psA")
                for jt in range(8):
                    for ic in range(2):
                        nc.tensor.matmul(
                            pso[:, IC[ic]],
                            r(v_sb[jt][:, h * 128 : (h + 1) * 128]),
                            r(E[jt][:, IC[ic]]),
                            start=(jt == 0),
                            stop=(jt == 7),
                            skip_group_check=True,
                        )
                for jt in range(8):
                    for ic in range(2):
                        q = q_loc + ic
                        mm = nc.tensor.matmul(
                            psc[half][:],
                            r(t_sb[:, 3 - q : 7 - q]),
                            r(E[jt][:, IC[ic]]),
                            start=(h % 2 == 0 and jt == 0 and ic == 0),
                            stop=(h % 2 == 1 and jt == 7 and ic == 1),
                            skip_group_check=True,
                        )
                        last_cs[half] = mm
                def o_copy():
                    nc.vector.tensor_copy(out=oT_sb[h][:], in_=pso[:])

                if defer_copy:
                    return o_copy
                o_copy()
                return None

            def normalize_half(half):
                # reciprocal of this half's colsums (overlaps later PE work)
                with nc.allow_low_precision(
                    reason="softmax denom reciprocal rounded to f32r"
                ):
                    nc.vector.reciprocal(out=csr[half][:], in_=psc[half][:])

            def broadcast_half(half):
                for hh in range(2):
                    h = half * 2 + hh
                    bc = ps_main.tile([128, HW], F32, tag="psA", name="psA")
                    for ic in range(2):
                        q = hh * 2 + ic
                        nc.tensor.matmul(
                            bc[:, IC[ic]],
                            r(u_sb[:, (3 - q) * 128 : (4 - q) * 128]),
                            r(csr[half][:]),
                        )
                    nc.vector.tensor_tensor(
                        out=oT_sb[h][:], in0=oT_sb[h][:], in1=bc[:], op=MUL
                    )

            attention_head(0)
            copy1 = attention_head(1, defer_copy=True)
            normalize_half(0)  # recip01 enters the DVE queue first
            copy1()
            attention_head(2)
            broadcast_half(0)  # csr01 long ready; no PE stall
            copy3 = attention_head(3, defer_copy=True)
            normalize_half(1)  # recip23 on DVE while PE does bres
            copy3()

            # ---- bres = bo + Wo^T @ bv (v-bias folded through out proj);
            # pinned after the last colsum so these tiny matmuls occupy the
            # PE exactly while recip23 runs.
            for kc in range(4):
                psb = ps_cs.tile([128, 1], F32, tag="psC", name="psB")
                for km in range(4):
                    bres_l = wo_sb[km][:, kc * 128 : (kc + 1) * 128]
                    bres_r = bv_sb[km][:]
                    if not BF16:
                        bres_l = bres_l.bitcast(F32)
                        bres_r = bres_r.bitcast(F32)
                    mm = nc.tensor.matmul(
                        psb[:],
                        bres_l,
                        bres_r,
                        start=(km == 0),
                        stop=(km == 3),
                    )
                    if km == 0:
                        add_dep_helper(
                            mm.ins,
                            last_cs[1].ins,
                            reason="pin bres into the recip23 window",
                        )
                nc.vector.tensor_add(
                    out=bres_sb[kc][:], in0=psb[:], in1=bo_sb[kc][:]
                )

            # ---- phase D part 1: kc0/kc1 accumulate heads 0-1 while
            # recip23 still runs on DVE (their oT are already normalized)
            def d_matmuls(ps, kc, kms, start_km, stop_km, pin=False):
                for km in kms:
                    for ic in range(2):
                        mm = nc.tensor.matmul(
                            ps[:, IC[ic]],
                            r(wo_sb[km][:, kc * 128 : (kc + 1) * 128]),
                            r(oT_sb[km][:, IC[ic]]),
                            start=(km == start_km),
                            stop=(km == stop_km),
                            skip_group_check=True,
                        )
                        if pin and km == kms[0] and ic == 0:
                            add_dep_helper(
                                mm.ins,
                                last_cs[1].ins,
                                reason="pin D part1 into the recip23 window",
                            )

            def d_finish(ps, kc):
                for ic in range(2):
                    ot = outp.tile([128, 512], F32, tag="out", name="out")
                    # out = (psum + bres) + x_residual in one DVE op
                    nc.vector.scalar_tensor_tensor(
                        out=ot[:],
                        in0=ps[:, IC[ic]],
                        scalar=bres_sb[kc][:],
                        in1=xf_sb[kc][:, IC[ic]],
                        op0=ADD,
                        op1=ADD,
                    )
                    nc.sync.dma_start(
                        out=out[kc * 128 : (kc + 1) * 128, IC[ic]], in_=ot[:]
                    )

            psD = {}
            for kc in range(2):
                psD[kc] = ps_main.tile([128, HW], F32, tag="psA", name="psA")
                d_matmuls(psD[kc], kc, [0, 1], start_km=0, stop_km=3)

            broadcast_half(1)

            for kc in range(2):
                d_matmuls(psD[kc], kc, [2, 3], start_km=0, stop_km=3)
                d_finish(psD[kc], kc)
            for kc in range(2, 4):
                ps = ps_main.tile([128, HW], F32, tag="psA", name="psA")
                d_matmuls(ps, kc, [0, 1, 2, 3], start_km=0, stop_km=3)
                d_finish(ps, kc)

    _split_multi_waits(nc)
    return nc


_NC_CACHE = {}


def _get_nc():
    if "nc" not in _NC_CACHE:
        _NC_CACHE["nc"] = build_attention_nc()
    return _NC_CACHE["nc"]


def _band_consts():
    tb = np.zeros((128, 7), dtype=mybir.dt.np(MMDT))
    tb[:, 3] = 1.0
    ub = np.zeros((4, 7 * 128), dtype=np.float32)
    for k in range(4):
        ub[k, (3 - k) * 128 : (4 - k) * 128] = 1.0
    return tb, ub


def run_sharded(x, Wp, bp, Wo, bo, **spmd_kwargs):
    """Shard over batch, run on cores 0-7, gather.  Returns ([B,C,H,W], res)."""
    mmnp = mybir.dt.np(MMDT)
    x = np.ascontiguousarray(x, dtype=np.float32)
    xbh = x.astype(mmnp)
    Wp = np.ascontiguousarray(Wp, dtype=np.float32).astype(mmnp)
    bpf_arr = np.ascontiguousarray(bp, dtype=np.float32).reshape(OCH, 1)
    bp = bpf_arr.astype(mmnp)
    Wo = np.ascontiguousarray(Wo, dtype=np.float32).astype(mmnp)
    bo = np.ascontiguousarray(bo, dtype=np.float32).reshape(C, 1)

    nc = _get_nc()
    tb, ub = _band_consts()
    in_maps = []
    for b in range(B):
        in_maps.append(
            {
                "xb": xbh[b].reshape(C, HW),
                "xf": x[b].reshape(C, HW),
                "wp": Wp,
                "bp": bp,
                "wo": Wo,
                "bo": bo,
                "bpf": bpf_arr,
                "tband": tb,
                "uband": ub,
            }
        )
    res = run_bass_kernel_spmd(nc, in_maps, core_ids=list(range(B)), **spmd_kwargs)
    h = w = int(np.sqrt(HW))
    out = np.stack([res.results[b]["out"].reshape(C, h, w) for b in range(B)])
    return out, res


def kernel(x, Wp, bp, Wo, bo):
    out, _ = run_sharded(x, Wp, bp, Wo, bo)
    return out



# revision 16
# speedup vs baseline: 1.0277x; 1.0277x over previous
"""AttentionBlock kernel for 8x Trainium2 NeuronCores.

Strategy: data-parallel over batch (B=8 -> 1 batch element per core), with
fp8-e4m3 DoubleRow matmuls (K=256 contraction per instruction = 2x the
FLOPs/column of bf16) everywhere the numerics allow, bf16 only for the
score matmuls (exp amplifies q/k quantization noise; fp8 scores fail the
2e-2 gate, bf16 scores measure ~1.3e-2 in simulation).

Per-core layout (channel-major, "transposed", no on-chip transposes):

  x8  [128, 2, 1024] fp8 pairs   (contraction rows c = i*256 + s*128 + p)
  wp8 [128, 2, 1536] fp8 pairs   (host-reordered columns [Q|K|V], x16 scale)
  A:  q/k^T[d, tok]  = wp8^T x8 (fp8 DR), DVE adds 16*bq to q (k-bias
      cancels in softmax and is dropped)
  B:  v[tok, och]    = x8^T wp8_V (fp8 DR), Pool copies psum -> v8 pairs
  S:  S^T[j, i]      = kT^T qT per j-tile (bf16)
  E = exp(S * scale/256 - 4) on ScalarE, fp8 out, pair tiles (the -4 bias
      keeps e^s under fp8-e4m3 max 240 and cancels in softmax)
  AV: O^T[d, i]     += v8^T E (fp8 DR); CS: colsums via banded-ones fp8 DR
      matmuls into a [2, 512] psum tile per head
  recip (DVE f32r), broadcast via u-band matmul (rows scaled 1/16), norm:
      Pool copies O psum -> bf16, DVE multiplies by broadcast recip -> fp8
  D:  out^T[c, tok]  = wo8^T oT8 (fp8 DR) + (bres + x) via one STT op,
      where bres = bo + Wo^T bv is precomputed on the host (sum_j P = 1).

Schedule: ScalarE runs the 32 exp ops back to back (~30us); the PE stream
is emitted so scores of head h+1 and AV/CS of head h interleave inside
head h's exp window.  All input DMAs trigger from the Pool queue (cheap
descriptor config), x8 from the SP queue, outputs from SP at the tail.
"""

import sys

sys.path.insert(0, "/opt/trn_rl_repo")

import numpy as np
import ml_dtypes

import concourse.bass as bass
import concourse.tile as tile
import concourse.mybir as mybir
from concourse.bass_utils import run_bass_kernel_spmd

B, C, HW = 8, 512, 1024
NH, DK = 4, 128
SCALE = float(DK) ** -0.5
WPS = 16.0  # host pre-scale of Wp into fp8 normal range
EXP_BIAS = -4.0

F32 = mybir.dt.float32
F32R = mybir.dt.float32r
BF16 = mybir.dt.bfloat16
F8 = mybir.dt.float8e4
DR = mybir.MatmulPerfMode.DoubleRow

# ---------------------------------------------------------------------------
# Walrus in this container supports only ONE embedded sync-wait per
# instruction; Tile emits multi-wait instructions, so rewrite each into
# single-wait NoOps + the instruction keeping its last wait.
# ---------------------------------------------------------------------------
_wsplit_counter = [0]


def _split_multi_waits(nc):
    for fn in nc.m.functions:
        for blk in fn.blocks:
            insts = blk.instructions
            if not insts:
                continue
            new = []
            changed = False
            for inst in insts:
                si = inst.sync_info
                waits = list(si.on_wait) if si is not None and si.on_wait else []
                if len(waits) > 1:
                    changed = True
                    for w in waits[:-1]:
                        _wsplit_counter[0] += 1
                        nop = mybir.InstNoOp(
                            name=f"WSPLIT-{_wsplit_counter[0]}",
                            ins=[],
                            outs=[],
                            engine=inst.engine,
                        )
                        nop.sync_info = mybir.SyncInfo(on_wait=[w], on_update=[])
                        nc.register_instruction(nop, overwrite=True)
                        new.append(nop)
                    inst.sync_info = mybir.SyncInfo(
                        on_wait=[waits[-1]], on_update=list(si.on_update or [])
                    )
                new.append(inst)
            if changed:
                blk.instructions = new


def build_attention_nc():
    nc = bass.Bass("TRN2")
    x8d = nc.dram_tensor("x8", [C, HW], F8, kind="ExternalInput")
    x16d = nc.dram_tensor("x16", [C, HW], BF16, kind="ExternalInput")
    wpd = nc.dram_tensor("wp", [C, 3 * C], F8, kind="ExternalInput")
    wod = nc.dram_tensor("wo", [C, C], F8, kind="ExternalInput")
    bqd = nc.dram_tensor("bq", [C, 1], F32, kind="ExternalInput")
    bresd = nc.dram_tensor("bres", [C, 1], F32, kind="ExternalInput")
    t2d = nc.dram_tensor("t2", [128, 512], F8, kind="ExternalInput")
    u2d = nc.dram_tensor("u2", [2, 256], F32R, kind="ExternalInput")
    outd = nc.dram_tensor("out", [C, HW], F32, kind="ExternalOutput")

    x8d, x16d, wpd, wod, bqd, bresd, t2d, u2d, outd = (
        t.ap() for t in (x8d, x16d, wpd, wod, bqd, bresd, t2d, u2d, outd)
    )

    EXP = mybir.ActivationFunctionType.Exp
    ADD = mybir.AluOpType.add
    MUL = mybir.AluOpType.mult
    IC = [slice(0, 512), slice(512, 1024)]

    with tile.TileContext(nc) as tc:
        with (
            tc.tile_pool(name="persist", bufs=1) as persist,
            tc.tile_pool(name="epool", bufs=8) as epool,
            tc.tile_pool(name="o16p", bufs=2) as o16p,
            tc.tile_pool(name="outp", bufs=2) as outp,
            tc.tile_pool(name="psM", bufs=2, space="PSUM") as psM,
            tc.tile_pool(name="psAV", bufs=1, space="PSUM") as psAV,
            tc.tile_pool(name="psCS", bufs=1, space="PSUM") as psCS,
            tc.tile_pool(name="psBC", bufs=1, space="PSUM") as psBC,
        ):
            # ---- persistent SBUF tensors -------------------------------
            x8_sb = [persist.tile([128, 2048], F8, tag=f"x8{i}", name=f"x8{i}") for i in range(2)]
            xb_sb = [persist.tile([128, HW], BF16, tag=f"xb{i}", name=f"xb{i}") for i in range(4)]
            wp_sb = [persist.tile([128, 2 * 1536], F8, tag=f"wp{i}", name=f"wp{i}") for i in range(2)]
            wo_sb = [persist.tile([128, 1024], F8, tag=f"wo{i}", name=f"wo{i}") for i in range(2)]
            qk_sb = [persist.tile([128, HW], BF16, tag=f"qk{i}", name=f"qk{i}") for i in range(8)]
            v8_sb = [persist.tile([128, 1024], F8, tag=f"v{i}", name=f"v{i}") for i in range(4)]
            oT8_sb = [persist.tile([128, 2048], F8, tag=f"oT{i}", name=f"oT{i}") for i in range(2)]
            bq_sb = [persist.tile([128, 1], F32, tag=f"bq{i}", name=f"bq{i}") for i in range(4)]
            bres_sb = [persist.tile([128, 1], F32, tag=f"br{i}", name=f"br{i}") for i in range(4)]
            csr_sb = [persist.tile([2, 512], F32R, tag=f"cs{i}", name=f"cs{i}") for i in range(4)]
            t2_sb = persist.tile([128, 512], F8, tag="t2", name="t2")
            u2_sb = persist.tile([2, 256], F32R, tag="u2", name="u2")
            warm_sb = persist.tile([1, 2], F32, tag="warm", name="warm")
            ebias_sb = persist.tile([128, 1], F32, tag="ebias", name="ebias")

            x83 = [t[:].rearrange("p (s n) -> p s n", s=2) for t in x8_sb]
            wp3 = [t[:].rearrange("p (s n) -> p s n", s=2) for t in wp_sb]
            wo3 = [t[:].rearrange("p (s n) -> p s n", s=2) for t in wo_sb]
            v83 = [t[:].rearrange("p (s n) -> p s n", s=2) for t in v8_sb]
            oT83 = [t[:].rearrange("p (s n) -> p s n", s=2) for t in oT8_sb]
            # two banded-ones selector weights (M=128): out row 0 / row 1
            t23 = [
                t2_sb[:, 0:256].rearrange("p (s n) -> p s n", s=2),
                t2_sb[:, 256:512].rearrange("p (s n) -> p s n", s=2),
            ]

            # ---- loads -------------------------------------------------
            # x8 on the SP queue; everything else on Pool (cheap DGE config),
            # ordered so phase A's first operands land first.
            for i in range(2):
                for s in range(2):
                    nc.sync.dma_start(
                        out=x8_sb[i][:, s * 1024 : (s + 1) * 1024],
                        in_=x8d[(2 * i + s) * 128 : (2 * i + s + 1) * 128, :],
                    )
            for c0, c1 in ((0, 512), (512, 1024)):  # Q block, then K block
                for i in range(2):
                    for s in range(2):
                        nc.gpsimd.dma_start(
                            out=wp_sb[i][:, s * 1536 + c0 : s * 1536 + c1],
                            in_=wpd[(2 * i + s) * 128 : (2 * i + s + 1) * 128, c0:c1],
                        )
            for h in range(NH):
                nc.gpsimd.dma_start(out=bq_sb[h], in_=bqd[h * 128 : (h + 1) * 128, 0:1])
            for i in range(2):  # V block
                for s in range(2):
                    nc.gpsimd.dma_start(
                        out=wp_sb[i][:, s * 1536 + 1024 : s * 1536 + 1536],
                        in_=wpd[(2 * i + s) * 128 : (2 * i + s + 1) * 128, 1024:1536],
                    )
            nc.gpsimd.dma_start(out=t2_sb, in_=t2d[:, :])
            nc.gpsimd.dma_start(out=u2_sb, in_=u2d[:, :])
            for m in range(2):
                for s in range(2):
                    nc.gpsimd.dma_start(
                        out=wo_sb[m][:, s * 512 : (s + 1) * 512],
                        in_=wod[(2 * m + s) * 128 : (2 * m + s + 1) * 128, :],
                    )
            for kc in range(4):
                nc.gpsimd.dma_start(out=xb_sb[kc], in_=x16d[kc * 128 : (kc + 1) * 128, :])
                nc.gpsimd.dma_start(out=bres_sb[kc], in_=bresd[kc * 128 : (kc + 1) * 128, 0:1])

            # ---- preload the Exp activation table during the DMA wait --
            nc.vector.memset(warm_sb[:], 0.0)
            nc.vector.memset(ebias_sb[:], EXP_BIAS)
            nc.scalar.activation(
                out=warm_sb[0:1, 0:1],
                in_=warm_sb[0:1, 1:2],
                func=EXP,
                bias=ebias_sb[0:1],
            )

            # ---- emission helpers --------------------------------------
            def proj_qk(h, qk):
                """q/k^T [128 d, 1024 tok] for head h (fp8 DR); DVE copy."""
                ps = psM.tile([128, HW], F32, tag="psM", name="psM")
                off = qk * 512 + h * 128
                for i in range(2):
                    for ic in range(2):
                        nc.tensor.matmul(
                            ps[:, IC[ic]],
                            wp3[i][:, :, off : off + 128],
                            x83[i][:, :, IC[ic]],
                            start=(i == 0),
                            stop=(i == 1),
                            perf_mode=DR,
                        )
                if qk == 0:
                    nc.vector.tensor_scalar_add(
                        out=qk_sb[h * 2][:], in0=ps[:], scalar1=bq_sb[h][:]
                    )
                else:
                    nc.vector.tensor_copy(out=qk_sb[h * 2 + 1][:], in_=ps[:])

            def proj_v(jp):
                """v [tok, och] for token pair-tile jp (fp8 DR); Pool copy."""
                ps = psM.tile([128, HW], F32, tag="psM", name="psM")
                for s in range(2):
                    jt = jp * 2 + s
                    for i in range(2):
                        nc.tensor.matmul(
                            ps[:, IC[s]],
                            x83[i][:, :, jt * 128 : (jt + 1) * 128],
                            wp3[i][:, :, 1024:1536],
                            start=(i == 0),
                            stop=(i == 1),
                            perf_mode=DR,
                            skip_group_check=True,
                        )
                nc.scalar.copy(out=v8_sb[jp][:], in_=ps[:])

            e_tiles = {}  # (h, jp) -> epool tile

            def scores(h, jt):
                """S^T [128 j, 1024 i] (bf16) + exp -> E fp8 pair slot."""
                qT, kT = qk_sb[h * 2], qk_sb[h * 2 + 1]
                ps = psM.tile([128, HW], F32, tag="psM", name="psM")
                for ic in range(2):
                    nc.tensor.matmul(
                        ps[:, IC[ic]],
                        kT[:, jt * 128 : (jt + 1) * 128],
                        qT[:, IC[ic]],
                    )
                jp, s = jt // 2, jt % 2
                if s == 0:
                    e_tiles[(h, jp)] = epool.tile([128, 2048], F8, tag="E", name="E")
                nc.scalar.activation(
                    out=e_tiles[(h, jp)][:, s * 1024 : (s + 1) * 1024],
                    in_=ps[:],
                    func=EXP,
                    scale=SCALE / (WPS * WPS),
                    bias=ebias_sb[:],
                )

            av_ps = {}  # h -> psAV tile
            cs_ps = {}  # h -> psCS tile

            def av_cs(h, jp):
                """AV + colsum accumulation for (head h, token pair jp)."""
                e3 = e_tiles[(h, jp)][:].rearrange("p (s n) -> p s n", s=2)
                if jp == 0:
                    av_ps[h] = psAV.tile([128, HW], F32, tag="psAV", name="psAV")
                    cs_ps[h] = psCS.tile([128, 512], F32, tag="psCS", name="psCS")
                for ic in range(2):
                    nc.tensor.matmul(
                        av_ps[h][:, IC[ic]],
                        v83[jp][:, :, h * 128 : (h + 1) * 128],
                        e3[:, :, IC[ic]],
                        start=(jp == 0),
                        stop=(jp == 3),
                        perf_mode=DR,
                        skip_group_check=True,
                    )
                for ic in range(2):
                    nc.tensor.matmul(
                        cs_ps[h][:],
                        t23[ic],
                        e3[:, :, IC[ic]],
                        start=(jp == 0 and ic == 0),
                        stop=(jp == 3 and ic == 1),
                        perf_mode=DR,
                        skip_group_check=True,
                    )

            def normalize(h):
                """recip (DVE) -> o_copy (DVE) -> per-ic broadcast + norm."""
                with nc.allow_low_precision(reason="softmax denom recip in f32r"):
                    nc.vector.reciprocal(out=csr_sb[h][:], in_=cs_ps[h][0:2, :])
                o16 = o16p.tile([128, HW], BF16, tag="o16", name="o16")
                nc.vector.tensor_copy(out=o16[:], in_=av_ps[h][:])
                for ic in range(2):
                    bc = psBC.tile([128, 512], F32, tag="psBC", name="psBC")
                    nc.tensor.matmul(
                        bc[:],
                        u2_sb[:, (1 - ic) * 128 : (2 - ic) * 128],
                        csr_sb[h][:],
                    )
                    nc.vector.tensor_tensor(
                        out=oT8_sb[h // 2][:, (h % 2) * 1024 + ic * 512 : (h % 2) * 1024 + (ic + 1) * 512],
                        in0=o16[:, IC[ic]],
                        in1=bc[:],
                        op=MUL,
                    )

            d_ps = {}

            def proj_out_mm(kc, mlist):
                """output projection accumulation (fp8 DR)."""
                if kc not in d_ps:
                    d_ps[kc] = psM.tile([128, HW], F32, tag="psM", name="psM")
                ps = d_ps[kc]
                for m in mlist:
                    for ic in range(2):
                        nc.tensor.matmul(
                            ps[:, IC[ic]],
                            wo3[m][:, :, kc * 128 : (kc + 1) * 128],
                            oT83[m][:, :, IC[ic]],
                            start=(m == 0),
                            stop=(m == 1),
                            perf_mode=DR,
                            skip_group_check=True,
                        )

            def finish(kc, engine):
                """(psum + bres) + x residual, one STT op per half; DMA out."""
                ps = d_ps[kc]
                ot = outp.tile([128, HW], F32, tag="out", name="out")
                for ic in range(2):
                    engine.scalar_tensor_tensor(
                        out=ot[:, IC[ic]],
                        in0=ps[:, IC[ic]],
                        scalar=bres_sb[kc][:],
                        in1=xb_sb[kc][:, IC[ic]],
                        op0=ADD,
                        op1=ADD,
                    )
                nc.sync.dma_start(out=outd[kc * 128 : (kc + 1) * 128, :], in_=ot[:])

            # ---- emission schedule -------------------------------------
            proj_qk(0, 0)
            proj_qk(0, 1)
            scores(0, 0)
            scores(0, 1)
            proj_qk(1, 0)
            proj_qk(1, 1)
            scores(0, 2)
            scores(0, 3)
            proj_v(0)
            proj_v(1)
            scores(0, 4)
            scores(0, 5)
            proj_v(2)
            proj_v(3)
            scores(0, 6)
            scores(0, 7)
            proj_qk(2, 0)
            proj_qk(2, 1)
            proj_qk(3, 0)
            proj_qk(3, 1)

            for h in range(NH):
                for jp in range(4):
                    if h < 3:
                        scores(h + 1, jp * 2)
                        scores(h + 1, jp * 2 + 1)
                    av_cs(h, jp)
                    if h == 3 and jp == 2:
                        # main-pool psum tiles are free once exp(3) drains;
                        # start the output projection early for heads 0-1
                        proj_out_mm(0, [0])
                        proj_out_mm(1, [0])
                normalize(h)

            proj_out_mm(0, [1])
            proj_out_mm(1, [1])
            finish(0, nc.vector)
            finish(1, nc.vector)
            proj_out_mm(2, [0, 1])
            finish(2, nc.vector)
            proj_out_mm(3, [0, 1])
            finish(3, nc.vector)

    _split_multi_waits(nc)
    return nc


_NC_CACHE = {}


def _get_nc():
    if "nc" not in _NC_CACHE:
        _NC_CACHE["nc"] = build_attention_nc()
    return _NC_CACHE["nc"]


def _host_consts():
    f8 = ml_dtypes.float8_e4m3
    # selector weights [p, (half, s, 128)]: half ic has its column ic set,
    # in both k-subtile slots, so matmul output row ic gets the colsum
    t2 = np.zeros((128, 512), dtype=f8)
    for s in range(2):
        t2[:, s * 128 + 0] = 1.0
        t2[:, 256 + s * 128 + 1] = 1.0
    u2 = np.zeros((2, 256), dtype=np.float32)
    u2[0, 128:256] = 1.0 / WPS
    u2[1, 0:128] = 1.0 / WPS
    return t2, u2


def _prep_weights(Wp, bp, Wo, bo):
    """Reorder Wp/bp columns to [Q|K|V] blocks, pre-scale by WPS, cast fp8;
    fold the v bias through the output projection on the host."""
    f8 = ml_dtypes.float8_e4m3
    Wp = np.ascontiguousarray(Wp, dtype=np.float32)
    bp = np.ascontiguousarray(bp, dtype=np.float32).reshape(-1)
    Wo = np.ascontiguousarray(Wo, dtype=np.float32)
    bo = np.ascontiguousarray(bo, dtype=np.float32).reshape(-1)
    qcols = np.concatenate([np.arange(h * 384, h * 384 + 128) for h in range(NH)])
    kcols = qcols + 128
    vcols = qcols + 256
    order = np.concatenate([qcols, kcols, vcols])
    wp8 = (Wp[:, order] * WPS).astype(f8)
    bq = (bp[qcols] * WPS).astype(np.float32).reshape(C, 1)
    bres = (bo + Wo.T @ bp[vcols]).astype(np.float32).reshape(C, 1)
    wo8 = Wo.astype(f8)
    return wp8, bq, wo8, bres


def run_sharded(x, Wp, bp, Wo, bo, **spmd_kwargs):
    """Shard over batch, run on cores 0-7, gather.  Returns ([B,C,H,W], res)."""
    f8 = ml_dtypes.float8_e4m3
    x = np.ascontiguousarray(x, dtype=np.float32).reshape(B, C, HW)
    wp8, bq, wo8, bres = _prep_weights(Wp, bp, Wo, bo)
    t2, u2 = _host_consts()
    x8 = x.astype(f8)
    x16 = x.astype(ml_dtypes.bfloat16)

    nc = _get_nc()
    in_maps = []
    for b in range(B):
        in_maps.append(
            {
                "x8": x8[b],
                "x16": x16[b],
                "wp": wp8,
                "wo": wo8,
                "bq": bq,
                "bres": bres,
                "t2": t2,
                "u2": u2,
            }
        )
    res = run_bass_kernel_spmd(nc, in_maps, core_ids=list(range(B)), **spmd_kwargs)
    h = w = int(np.sqrt(HW))
    out = np.stack([res.results[b]["out"].reshape(C, h, w) for b in range(B)])
    return out, res


def kernel(x, Wp, bp, Wo, bo):
    out, _ = run_sharded(x, Wp, bp, Wo, bo)
    return out


# revision 33
# speedup vs baseline: 1.0361x; 1.0082x over previous
"""AttentionBlock kernel for 8x Trainium2 NeuronCores.

Strategy: data-parallel over batch (B=8 -> 1 batch element per core), with
fp8-e4m3 DoubleRow matmuls (K=256 contraction per instruction = 2x the
FLOPs/column of bf16) everywhere the numerics allow, bf16 only for the
score matmuls (exp amplifies q/k quantization noise; fp8 scores fail the
2e-2 gate, bf16 scores measure ~1.3e-2 in simulation).

Per-core layout (channel-major, "transposed", no on-chip transposes):

  x8  [128, 2, 1024] fp8 pairs   (contraction rows c = i*256 + s*128 + p)
  wp8 [128, 2, 1536] fp8 pairs   (host-reordered columns [Q|K|V], x16 scale)
  A:  q/k^T[d, tok]  = wp8^T x8 (fp8 DR), DVE adds 16*bq to q (k-bias
      cancels in softmax and is dropped)
  B:  v[tok, och]    = x8^T wp8_V (fp8 DR), Pool copies psum -> v8 pairs
  S:  S^T[j, i]      = kT^T qT per j-tile (bf16)
  E = exp(S * scale/256 - 4) on ScalarE, fp8 out, pair tiles (the -4 bias
      keeps e^s under fp8-e4m3 max 240 and cancels in softmax)
  AV: O^T[d, i]     += v8^T E (fp8 DR); CS: colsums via banded-ones fp8 DR
      matmuls into a [2, 512] psum tile per head
  recip (DVE f32r), broadcast via u-band matmul (rows scaled 1/16), norm:
      Pool copies O psum -> bf16, DVE multiplies by broadcast recip -> fp8
  D:  out^T[c, tok]  = wo8^T oT8 (fp8 DR) + (bres + x) via one STT op,
      where bres = bo + Wo^T bv is precomputed on the host (sum_j P = 1).

Schedule: ScalarE runs the 32 exp ops back to back (~30us); the PE stream
is emitted so scores of head h+1 and AV/CS of head h interleave inside
head h's exp window.  All input DMAs trigger from the Pool queue (cheap
descriptor config), x8 from the SP queue, outputs from SP at the tail.
"""

import sys

sys.path.insert(0, "/opt/trn_rl_repo")

import numpy as np
import ml_dtypes

import concourse.bass as bass
import concourse.tile as tile
import concourse.mybir as mybir
from concourse.bass_utils import run_bass_kernel_spmd

B, C, HW = 8, 512, 1024
NH, DK = 4, 128
SCALE = float(DK) ** -0.5
WPS = 16.0  # host pre-scale of Wp into fp8 normal range
EXP_BIAS = -4.0

F32 = mybir.dt.float32
F32R = mybir.dt.float32r
BF16 = mybir.dt.bfloat16
F8 = mybir.dt.float8e4
DR = mybir.MatmulPerfMode.DoubleRow

# ---------------------------------------------------------------------------
# Walrus in this container supports only ONE embedded sync-wait per
# instruction; Tile emits multi-wait instructions, so rewrite each into
# single-wait NoOps + the instruction keeping its last wait.
# ---------------------------------------------------------------------------
_wsplit_counter = [0]


def _split_multi_waits(nc):
    for fn in nc.m.functions:
        for blk in fn.blocks:
            insts = blk.instructions
            if not insts:
                continue
            new = []
            changed = False
            for inst in insts:
                si = inst.sync_info
                waits = list(si.on_wait) if si is not None and si.on_wait else []
                if len(waits) > 1:
                    changed = True
                    for w in waits[:-1]:
                        _wsplit_counter[0] += 1
                        nop = mybir.InstNoOp(
                            name=f"WSPLIT-{_wsplit_counter[0]}",
                            ins=[],
                            outs=[],
                            engine=inst.engine,
                        )
                        nop.sync_info = mybir.SyncInfo(on_wait=[w], on_update=[])
                        nc.register_instruction(nop, overwrite=True)
                        new.append(nop)
                    inst.sync_info = mybir.SyncInfo(
                        on_wait=[waits[-1]], on_update=list(si.on_update or [])
                    )
                new.append(inst)
            if changed:
                blk.instructions = new


def build_attention_nc():
    nc = bass.Bass("TRN2")
    # all per-core inputs come in partition-major [128, ...] host layouts so
    # each tensor is ONE contiguous DMA (triggers cost ~600ns each, serially,
    # per queue -- and the GpSimd queue boots ~6us late on this runtime)
    x8d = nc.dram_tensor("x8", [128, 4 * HW], F8, kind="ExternalInput")
    x16d = nc.dram_tensor("x16", [128, 4 * HW], BF16, kind="ExternalInput")
    wpd = nc.dram_tensor("wp", [128, 4 * 1536], F8, kind="ExternalInput")
    wod = nc.dram_tensor("wo", [128, 4 * C], F8, kind="ExternalInput")
    bqd = nc.dram_tensor("bq", [128, 4], F32, kind="ExternalInput")
    bresd = nc.dram_tensor("bres", [128, 4], F32, kind="ExternalInput")
    t2d = nc.dram_tensor("t2", [128, 512], F8, kind="ExternalInput")
    u2d = nc.dram_tensor("u2", [2, 256], F32R, kind="ExternalInput")
    outd = nc.dram_tensor("out", [C, HW], F32, kind="ExternalOutput")

    x8d, x16d, wpd, wod, bqd, bresd, t2d, u2d, outd = (
        t.ap() for t in (x8d, x16d, wpd, wod, bqd, bresd, t2d, u2d, outd)
    )

    EXP = mybir.ActivationFunctionType.Exp
    ADD = mybir.AluOpType.add
    MUL = mybir.AluOpType.mult
    IC = [slice(0, 512), slice(512, 1024)]

    with tile.TileContext(nc) as tc:
        with (
            tc.tile_pool(name="persist", bufs=1) as persist,
            tc.tile_pool(name="epool", bufs=8) as epool,
            tc.tile_pool(name="o16p", bufs=2) as o16p,
            tc.tile_pool(name="outp", bufs=2) as outp,
            tc.tile_pool(name="psM", bufs=2, space="PSUM") as psM,
            tc.tile_pool(name="psAV", bufs=1, space="PSUM") as psAV,
            tc.tile_pool(name="psCS", bufs=2, space="PSUM") as psCS,
        ):
            # ---- persistent SBUF tensors -------------------------------
            x8_sb = persist.tile([128, 4 * HW], F8, tag="x8", name="x8")
            x16_sb = persist.tile([128, 4 * HW], BF16, tag="x16", name="x16")
            wp_sb = persist.tile([128, 4 * 1536], F8, tag="wp", name="wp")
            wo_sb = persist.tile([128, 4 * C], F8, tag="wo", name="wo")
            qk_sb = [persist.tile([128, HW], BF16, tag=f"qk{i}", name=f"qk{i}") for i in range(8)]
            v8_sb = [persist.tile([128, 1024], F8, tag=f"v{i}", name=f"v{i}") for i in range(4)]
            oT8_sb = [persist.tile([128, 2048], F8, tag=f"oT{i}", name=f"oT{i}") for i in range(2)]
            bq_sb = persist.tile([128, 4], F32, tag="bq", name="bq")
            bres_sb = persist.tile([128, 4], F32, tag="br", name="br")
            csr_sb = [persist.tile([2, 512], F32R, tag=f"cs{i}", name=f"cs{i}") for i in range(4)]
            t2_sb = persist.tile([128, 512], F8, tag="t2", name="t2")
            u2_sb = persist.tile([2, 256], F32R, tag="u2", name="u2")
            warm_sb = persist.tile([1, 2], F32, tag="warm", name="warm")
            ebias_sb = persist.tile([128, 1], F32, tag="ebias", name="ebias")

            # [p, c-subtile, n] views; DR slices take 2 consecutive subtiles
            x84 = x8_sb[:].rearrange("p (c n) -> p c n", c=4)
            xb4 = x16_sb[:].rearrange("p (c n) -> p c n", c=4)
            wp4 = wp_sb[:].rearrange("p (c n) -> p c n", c=4)
            wo4 = wo_sb[:].rearrange("p (c n) -> p c n", c=4)
            v83 = [t[:].rearrange("p (s n) -> p s n", s=2) for t in v8_sb]
            oT83 = [t[:].rearrange("p (s n) -> p s n", s=2) for t in oT8_sb]
            # two banded-ones selector weights (M=128): out row 0 / row 1
            t23 = [
                t2_sb[:, 0:256].rearrange("p (s n) -> p s n", s=2),
                t2_sb[:, 256:512].rearrange("p (s n) -> p s n", s=2),
            ]

            # ---- loads: one DMA per tensor -----------------------------
            # critical path (x8, wp) + x16 on SP; small consts on the DVE
            # and Act queues (their first real work starts late anyway);
            # nothing on GpSimd (it boots ~6us into the kernel).
            nc.sync.dma_start(out=x8_sb, in_=x8d[:, :])
            nc.sync.dma_start(out=wp_sb, in_=wpd[:, :])
            nc.sync.dma_start(out=x16_sb, in_=x16d[:, :])
            nc.scalar.dma_start(out=bq_sb, in_=bqd[:, :])
            nc.scalar.dma_start(out=t2_sb, in_=t2d[:, :])
            nc.scalar.dma_start(out=u2_sb, in_=u2d[:, :])
            nc.scalar.dma_start(out=wo_sb, in_=wod[:, :])
            nc.scalar.dma_start(out=bres_sb, in_=bresd[:, :])

            # ---- preload the Exp activation table during the DMA wait --
            nc.vector.memset(warm_sb[:], 0.0)
            nc.vector.memset(ebias_sb[:], EXP_BIAS)
            nc.scalar.activation(
                out=warm_sb[0:1, 0:1],
                in_=warm_sb[0:1, 1:2],
                func=EXP,
                bias=ebias_sb[0:1],
            )

            # ---- emission helpers --------------------------------------
            def proj_qk(h, qk):
                """q/k^T [128 d, 1024 tok] for head h (fp8 DR); DVE copy."""
                ps = psM.tile([128, HW], F32, tag="psM", name="psM")
                off = qk * 512 + h * 128
                for i in range(2):
                    for ic in range(2):
                        nc.tensor.matmul(
                            ps[:, IC[ic]],
                            wp4[:, 2 * i : 2 * i + 2, off : off + 128],
                            x84[:, 2 * i : 2 * i + 2, IC[ic]],
                            start=(i == 0),
                            stop=(i == 1),
                            perf_mode=DR,
                        )
                if qk == 0:
                    nc.vector.tensor_scalar_add(
                        out=qk_sb[h * 2][:], in0=ps[:], scalar1=bq_sb[:, h : h + 1]
                    )
                else:
                    nc.vector.tensor_copy(out=qk_sb[h * 2 + 1][:], in_=ps[:])

            def proj_v(jp):
                """v [tok, och] for token pair-tile jp (fp8 DR); Act copy."""
                ps = psM.tile([128, HW], F32, tag="psM", name="psM")
                for s in range(2):
                    jt = jp * 2 + s
                    for i in range(2):
                        nc.tensor.matmul(
                            ps[:, IC[s]],
                            x84[:, 2 * i : 2 * i + 2, jt * 128 : (jt + 1) * 128],
                            wp4[:, 2 * i : 2 * i + 2, 1024:1536],
                            start=(i == 0),
                            stop=(i == 1),
                            perf_mode=DR,
                            skip_group_check=True,
                        )
                nc.scalar.copy(out=v8_sb[jp][:], in_=ps[:])

            e_tiles = {}  # (h, jp) -> epool tile

            def scores(h, jt):
                """S^T [128 j, 1024 i] (bf16) + exp -> E fp8 pair slot."""
                qT, kT = qk_sb[h * 2], qk_sb[h * 2 + 1]
                ps = psM.tile([128, HW], F32, tag="psM", name="psM")
                for ic in range(2):
                    nc.tensor.matmul(
                        ps[:, IC[ic]],
                        kT[:, jt * 128 : (jt + 1) * 128],
                        qT[:, IC[ic]],
                    )
                jp, s = jt // 2, jt % 2
                if s == 0:
                    e_tiles[(h, jp)] = epool.tile([128, 2048], F8, tag="E", name="E")
                nc.scalar.activation(
                    out=e_tiles[(h, jp)][:, s * 1024 : (s + 1) * 1024],
                    in_=ps[:],
                    func=EXP,
                    scale=SCALE / (WPS * WPS),
                    bias=ebias_sb[:],
                )

            av_ps = {}  # h -> psAV tile
            cs_ps = {}  # h -> psCS tile

            def av_mm(h, jp):
                """AV accumulation for (head h, token pair jp) (fp8 DR)."""
                e3 = e_tiles[(h, jp)][:].rearrange("p (s n) -> p s n", s=2)
                if jp == 0:
                    av_ps[h] = psAV.tile([128, HW], F32, tag="psAV", name="psAV")
                for ic in range(2):
                    nc.tensor.matmul(
                        av_ps[h][:, IC[ic]],
                        v83[jp][:, :, h * 128 : (h + 1) * 128],
                        e3[:, :, IC[ic]],
                        start=(jp == 0),
                        stop=(jp == 3),
                        perf_mode=DR,
                        skip_group_check=True,
                    )

            def cs_mm(h, jp):
                """colsum accumulation (deferred one jp so the psc pool
                rotation never stalls the PE on the previous head's recip)."""
                e3 = e_tiles[(h, jp)][:].rearrange("p (s n) -> p s n", s=2)
                if jp == 0:
                    cs_ps[h] = psCS.tile([128, 512], F32, tag="psCS", name="psCS")
                for ic in range(2):
                    nc.tensor.matmul(
                        cs_ps[h][:],
                        t23[ic],
                        e3[:, :, IC[ic]],
                        start=(jp == 0 and ic == 0),
                        stop=(jp == 3 and ic == 1),
                        perf_mode=DR,
                        skip_group_check=True,
                    )

            o16_tiles = {}

            def o_copy_recip(h):
                """free the AV psum immediately, then start the (slow)
                reciprocal on DVE; neither blocks the PE stream."""
                o16 = o16p.tile([128, HW], BF16, tag="o16", name="o16")
                nc.vector.tensor_copy(out=o16[:], in_=av_ps[h][:])
                o16_tiles[h] = o16
                with nc.allow_low_precision(reason="softmax denom recip f32r"):
                    nc.vector.reciprocal(out=csr_sb[h][:], in_=cs_ps[h][0:2, :])

            def bcast_norm(h):
                """emitted one head later, when recip(h) is long done."""
                for ic in range(2):
                    bc = psCS.tile([128, 512], F32, tag="psCS", name="psCS")
                    nc.tensor.matmul(
                        bc[:],
                        u2_sb[:, (1 - ic) * 128 : (2 - ic) * 128],
                        csr_sb[h][:],
                    )
                    nc.vector.tensor_tensor(
                        out=oT8_sb[h // 2][:, (h % 2) * 1024 + ic * 512 : (h % 2) * 1024 + (ic + 1) * 512],
                        in0=o16_tiles[h][:, IC[ic]],
                        in1=bc[:],
                        op=MUL,
                    )

            d_ps = {}

            def proj_out_mm(kc, mlist):
                """output projection accumulation (fp8 DR)."""
                if kc not in d_ps:
                    d_ps[kc] = psM.tile([128, HW], F32, tag="psM", name="psM")
                ps = d_ps[kc]
                for m in mlist:
                    for ic in range(2):
                        nc.tensor.matmul(
                            ps[:, IC[ic]],
                            wo4[:, 2 * m : 2 * m + 2, kc * 128 : (kc + 1) * 128],
                            oT83[m][:, :, IC[ic]],
                            start=(m == 0),
                            stop=(m == 1),
                            perf_mode=DR,
                            skip_group_check=True,
                        )

            def finish(kc, engine):
                """(psum + bres) + x residual, one STT op per half; DMA out."""
                ps = d_ps[kc]
                ot = outp.tile([128, HW], F32, tag="out", name="out")
                for ic in range(2):
                    engine.scalar_tensor_tensor(
                        out=ot[:, IC[ic]],
                        in0=ps[:, IC[ic]],
                        scalar=bres_sb[:, kc : kc + 1],
                        in1=x16_sb[:, kc * 1024 + ic * 512 : kc * 1024 + (ic + 1) * 512],
                        op0=ADD,
                        op1=ADD,
                    )
                nc.sync.dma_start(out=outd[kc * 128 : (kc + 1) * 128, :], in_=ot[:])

            # ---- emission schedule -------------------------------------
            proj_qk(0, 0)
            proj_qk(0, 1)
            scores(0, 0)
            scores(0, 1)
            proj_qk(1, 0)
            proj_qk(1, 1)
            scores(0, 2)
            scores(0, 3)
            proj_v(0)
            proj_v(1)
            scores(0, 4)
            scores(0, 5)
            proj_v(2)
            proj_v(3)
            scores(0, 6)
            scores(0, 7)
            proj_qk(2, 0)
            proj_qk(2, 1)
            proj_qk(3, 0)
            proj_qk(3, 1)

            for h in range(NH):
                for jp in range(4):
                    if h < 3:
                        scores(h + 1, jp * 2)
                        scores(h + 1, jp * 2 + 1)
                    av_mm(h, jp)
                    if jp == 1:
                        cs_mm(h, 0)
                    if jp >= 1:
                        cs_mm(h, jp)
                    if jp == 2 and h >= 1:
                        bcast_norm(h - 1)
                    if h == 3 and jp == 2:
                        # main-pool psum tiles are free once exp(3) drains;
                        # start the output projection early for heads 0-1
                        proj_out_mm(0, [0])
                        proj_out_mm(1, [0])
                o_copy_recip(h)

            bcast_norm(3)
            proj_out_mm(0, [1])
            proj_out_mm(1, [1])
            finish(0, nc.vector)
            finish(1, nc.vector)
            proj_out_mm(2, [0, 1])
            finish(2, nc.vector)
            proj_out_mm(3, [0, 1])
            finish(3, nc.vector)

    _split_multi_waits(nc)
    return nc


_NC_CACHE = {}


def _get_nc():
    if "nc" not in _NC_CACHE:
        _NC_CACHE["nc"] = build_attention_nc()
    return _NC_CACHE["nc"]


def _host_consts():
    f8 = ml_dtypes.float8_e4m3
    # selector weights [p, (half, s, 128)]: half ic has its column ic set,
    # in both k-subtile slots, so matmul output row ic gets the colsum
    t2 = np.zeros((128, 512), dtype=f8)
    for s in range(2):
        t2[:, s * 128 + 0] = 1.0
        t2[:, 256 + s * 128 + 1] = 1.0
    u2 = np.zeros((2, 256), dtype=np.float32)
    u2[0, 128:256] = 1.0 / WPS
    u2[1, 0:128] = 1.0 / WPS
    return t2, u2


def _prep_weights(Wp, bp, Wo, bo):
    """Reorder Wp/bp columns to [Q|K|V] blocks, pre-scale by WPS, cast fp8;
    fold the v bias through the output projection on the host."""
    f8 = ml_dtypes.float8_e4m3
    Wp = np.ascontiguousarray(Wp, dtype=np.float32)
    bp = np.ascontiguousarray(bp, dtype=np.float32).reshape(-1)
    Wo = np.ascontiguousarray(Wo, dtype=np.float32)
    bo = np.ascontiguousarray(bo, dtype=np.float32).reshape(-1)
    qcols = np.concatenate([np.arange(h * 384, h * 384 + 128) for h in range(NH)])
    kcols = qcols + 128
    vcols = qcols + 256
    order = np.concatenate([qcols, kcols, vcols])
    wp8 = (Wp[:, order] * WPS).astype(f8)
    bq = (bp[qcols] * WPS).astype(np.float32).reshape(C, 1)
    bres = (bo + Wo.T @ bp[vcols]).astype(np.float32).reshape(C, 1)
    wo8 = Wo.astype(f8)
    return wp8, bq, wo8, bres


def _pmajor(a):
    """[512, N] -> [128, 4*N]: row c = cs*128 + p lands at [p, cs*N + n]."""
    n = a.shape[1]
    return np.ascontiguousarray(a.reshape(4, 128, n).transpose(1, 0, 2).reshape(128, 4 * n))


def run_sharded(x, Wp, bp, Wo, bo, **spmd_kwargs):
    """Shard over batch, run on cores 0-7, gather.  Returns ([B,C,H,W], res)."""
    f8 = ml_dtypes.float8_e4m3
    x = np.ascontiguousarray(x, dtype=np.float32).reshape(B, C, HW)
    wp8, bq, wo8, bres = _prep_weights(Wp, bp, Wo, bo)
    wp8 = _pmajor(wp8)
    wo8 = _pmajor(wo8)
    bq = _pmajor(bq)
    bres = _pmajor(bres)
    t2, u2 = _host_consts()
    x8 = np.stack([_pmajor(x[b].astype(f8)) for b in range(B)])
    x16 = np.stack([_pmajor(x[b].astype(ml_dtypes.bfloat16)) for b in range(B)])

    nc = _get_nc()
    in_maps = []
    for b in range(B):
        in_maps.append(
            {
                "x8": x8[b],
                "x16": x16[b],
                "wp": wp8,
                "wo": wo8,
                "bq": bq,
                "bres": bres,
                "t2": t2,
                "u2": u2,
            }
        )
    res = run_bass_kernel_spmd(nc, in_maps, core_ids=list(range(B)), **spmd_kwargs)
    h = w = int(np.sqrt(HW))
    out = np.stack([res.results[b]["out"].reshape(C, h, w) for b in range(B)])
    return out, res


def kernel(x, Wp, bp, Wo, bo):
    out, _ = run_sharded(x, Wp, bp, Wo, bo)
    return out


# revision 36
# speedup vs baseline: 1.0969x; 1.0587x over previous
"""AttentionBlock kernel for 8x Trainium2 NeuronCores.

Strategy: data-parallel over batch (B=8 -> 1 batch element per core), with
fp8-e4m3 DoubleRow matmuls (K=256 contraction per instruction = 2x the
FLOPs/column of bf16) everywhere the numerics allow, bf16 only for the
score matmuls (exp amplifies q/k quantization noise; fp8 scores fail the
2e-2 gate, bf16 scores measure ~1.3e-2 in simulation).

Per-core layout (channel-major, "transposed", no on-chip transposes):

  x8  [128, 2, 1024] fp8 pairs   (contraction rows c = i*256 + s*128 + p)
  wp8 [128, 2, 1536] fp8 pairs   (host-reordered columns [Q|K|V], x16 scale)
  A:  q/k^T[d, tok]  = wp8^T x8 (fp8 DR), DVE adds 16*bq to q (k-bias
      cancels in softmax and is dropped)
  B:  v[tok, och]    = x8^T wp8_V (fp8 DR), Pool copies psum -> v8 pairs
  S:  S^T[j, i]      = kT^T qT per j-tile (bf16)
  E = exp(S * scale/256 - 4) on ScalarE, fp8 out, pair tiles (the -4 bias
      keeps e^s under fp8-e4m3 max 240 and cancels in softmax)
  AV: O^T[d, i]     += v8^T E (fp8 DR); CS: colsums via banded-ones fp8 DR
      matmuls into a [2, 512] psum tile per head
  recip (DVE f32r), broadcast via u-band matmul (rows scaled 1/16), norm:
      Pool copies O psum -> bf16, DVE multiplies by broadcast recip -> fp8
  D:  out^T[c, tok]  = wo8^T oT8 (fp8 DR) + (bres + x) via one STT op,
      where bres = bo + Wo^T bv is precomputed on the host (sum_j P = 1).

Schedule: ScalarE runs the 32 exp ops back to back (~30us); the PE stream
is emitted so scores of head h+1 and AV/CS of head h interleave inside
head h's exp window.  All input DMAs trigger from the Pool queue (cheap
descriptor config), x8 from the SP queue, outputs from SP at the tail.
"""

import sys

sys.path.insert(0, "/opt/trn_rl_repo")

import numpy as np
import ml_dtypes

import concourse.bass as bass
import concourse.tile as tile
import concourse.mybir as mybir
from concourse.bass_utils import run_bass_kernel_spmd

B, C, HW = 8, 512, 1024
NH, DK = 4, 128
SCALE = float(DK) ** -0.5
WPS = 16.0  # host pre-scale of Wp into fp8 normal range
EXP_BIAS = -4.0

F32 = mybir.dt.float32
F32R = mybir.dt.float32r
BF16 = mybir.dt.bfloat16
F8 = mybir.dt.float8e4
DR = mybir.MatmulPerfMode.DoubleRow

# ---------------------------------------------------------------------------
# Walrus in this container supports only ONE embedded sync-wait per
# instruction; Tile emits multi-wait instructions, so rewrite each into
# single-wait NoOps + the instruction keeping its last wait.
# ---------------------------------------------------------------------------
_wsplit_counter = [0]


def _split_multi_waits(nc):
    for fn in nc.m.functions:
        for blk in fn.blocks:
            insts = blk.instructions
            if not insts:
                continue
            new = []
            changed = False
            for inst in insts:
                si = inst.sync_info
                waits = list(si.on_wait) if si is not None and si.on_wait else []
                if len(waits) > 1:
                    changed = True
                    for w in waits[:-1]:
                        _wsplit_counter[0] += 1
                        nop = mybir.InstNoOp(
                            name=f"WSPLIT-{_wsplit_counter[0]}",
                            ins=[],
                            outs=[],
                            engine=inst.engine,
                        )
                        nop.sync_info = mybir.SyncInfo(on_wait=[w], on_update=[])
                        nc.register_instruction(nop, overwrite=True)
                        new.append(nop)
                    inst.sync_info = mybir.SyncInfo(
                        on_wait=[waits[-1]], on_update=list(si.on_update or [])
                    )
                new.append(inst)
            if changed:
                blk.instructions = new


def build_attention_nc():
    nc = bass.Bass("TRN2")
    # partition-major host layouts: one contiguous DMA per tensor chunk
    x8d = nc.dram_tensor("x8", [128, 4 * HW], F8, kind="ExternalInput")
    x16d = nc.dram_tensor("x16", [128, 4 * HW], BF16, kind="ExternalInput")
    # wp block-major: [p, (block q/k/v, c-subtile, 512)]
    wpd = nc.dram_tensor("wp", [128, 3 * 4 * 512], F8, kind="ExternalInput")
    wod = nc.dram_tensor("wo", [128, 4 * C], F8, kind="ExternalInput")
    bqd = nc.dram_tensor("bq", [128, 4], F32, kind="ExternalInput")
    bresd = nc.dram_tensor("bres", [128, 4], F32, kind="ExternalInput")
    t2d = nc.dram_tensor("t2", [128, 512], F8, kind="ExternalInput")
    u2d = nc.dram_tensor("u2", [2, 256], F32R, kind="ExternalInput")
    outd = nc.dram_tensor("out", [C, HW], F32, kind="ExternalOutput")

    x8d, x16d, wpd, wod, bqd, bresd, t2d, u2d, outd = (
        t.ap() for t in (x8d, x16d, wpd, wod, bqd, bresd, t2d, u2d, outd)
    )

    EXP = mybir.ActivationFunctionType.Exp
    ADD = mybir.AluOpType.add
    MUL = mybir.AluOpType.mult
    IC = [slice(0, 512), slice(512, 1024)]

    with tile.TileContext(nc) as tc:
        with (
            tc.tile_pool(name="persist", bufs=1) as persist,
            tc.tile_pool(name="epool", bufs=8) as epool,
            tc.tile_pool(name="o16p", bufs=4) as o16p,
            tc.tile_pool(name="outp", bufs=2) as outp,
            tc.tile_pool(name="psM", bufs=2, space="PSUM") as psM,
            tc.tile_pool(name="psX", bufs=4, space="PSUM") as psX,
        ):
            # ---- persistent SBUF tensors -------------------------------
            x8_sb = persist.tile([128, 4 * HW], F8, tag="x8", name="x8")
            x16_sb = persist.tile([128, 4 * HW], BF16, tag="x16", name="x16")
            wp_sb = persist.tile([128, 6144], F8, tag="wp", name="wp")
            wo_sb = persist.tile([128, 4 * C], F8, tag="wo", name="wo")
            qk_sb = [persist.tile([128, HW], BF16, tag=f"qk{i}", name=f"qk{i}") for i in range(8)]
            v8_sb = [persist.tile([128, 1024], F8, tag=f"v{i}", name=f"v{i}") for i in range(4)]
            oT8_sb = [persist.tile([128, 2048], F8, tag=f"oT{i}", name=f"oT{i}") for i in range(2)]
            bq_sb = persist.tile([128, 4], F32, tag="bq", name="bq")
            bres_sb = persist.tile([128, 4], F32, tag="br", name="br")
            csr_sb = [persist.tile([2, 512], F32R, tag=f"cs{i}", name=f"cs{i}") for i in range(4)]
            t2_sb = persist.tile([128, 512], F8, tag="t2", name="t2")
            u2_sb = persist.tile([2, 256], F32R, tag="u2", name="u2")
            warm_sb = persist.tile([1, 2], F32, tag="warm", name="warm")
            ebias_sb = persist.tile([128, 1], F32, tag="ebias", name="ebias")

            x84 = x8_sb[:].rearrange("p (c n) -> p c n", c=4)
            # per-block [p, c-subtile, 512] views of wp
            wpb = [
                wp_sb[:, b * 2048 : (b + 1) * 2048].rearrange("p (c n) -> p c n", c=4)
                for b in range(3)
            ]
            wo4 = wo_sb[:].rearrange("p (c n) -> p c n", c=4)
            v83 = [t[:].rearrange("p (s n) -> p s n", s=2) for t in v8_sb]
            oT83 = [t[:].rearrange("p (s n) -> p s n", s=2) for t in oT8_sb]
            t23 = [
                t2_sb[:, 0:256].rearrange("p (s n) -> p s n", s=2),
                t2_sb[:, 256:512].rearrange("p (s n) -> p s n", s=2),
            ]

            # ---- loads (SP: critical path; Act: consts; GpSimd: late) --
            nc.sync.dma_start(out=x8_sb[:, 0:2048], in_=x8d[:, 0:2048])
            nc.sync.dma_start(out=x8_sb[:, 2048:4096], in_=x8d[:, 2048:4096])
            for b in range(3):  # Q, K, V blocks
                nc.sync.dma_start(
                    out=wp_sb[:, b * 2048 : (b + 1) * 2048],
                    in_=wpd[:, b * 2048 : (b + 1) * 2048],
                )
            nc.scalar.dma_start(out=bq_sb, in_=bqd[:, :])
            nc.scalar.dma_start(out=t2_sb, in_=t2d[:, :])
            nc.scalar.dma_start(out=u2_sb, in_=u2d[:, :])
            nc.scalar.dma_start(out=bres_sb, in_=bresd[:, :])
            nc.gpsimd.dma_start(out=x16_sb, in_=x16d[:, :])
            nc.gpsimd.dma_start(out=wo_sb, in_=wod[:, :])

            # ---- preload the Exp table while DMAs land -----------------
            nc.vector.memset(warm_sb[:], 0.0)
            nc.vector.memset(ebias_sb[:], EXP_BIAS)
            nc.scalar.activation(
                out=warm_sb[0:1, 0:1],
                in_=warm_sb[0:1, 1:2],
                func=EXP,
                bias=ebias_sb[0:1],
            )

            # ---- helpers ----------------------------------------------
            def proj_qk(h, qk):
                """q/k^T [128 d, 1024] per-ic psum halves (fp8 DR) + DVE cast."""
                for ic in range(2):
                    ps = psX.tile([128, 512], F32, tag="psX", name="psX")
                    for i in range(2):
                        nc.tensor.matmul(
                            ps[:],
                            wpb[qk][:, 2 * i : 2 * i + 2, h * 128 : (h + 1) * 128],
                            x84[:, 2 * i : 2 * i + 2, IC[ic]],
                            start=(i == 0),
                            stop=(i == 1),
                            perf_mode=DR,
                            skip_group_check=True,
                        )
                    if qk == 0:
                        nc.vector.tensor_scalar_add(
                            out=qk_sb[h * 2][:, IC[ic]],
                            in0=ps[:],
                            scalar1=bq_sb[:, h : h + 1],
                        )
                    else:
                        nc.vector.tensor_copy(out=qk_sb[h * 2 + 1][:, IC[ic]], in_=ps[:])

            def proj_v(jt):
                """v [tok, och] one j-tile (fp8 DR) + Act copy to fp8."""
                ps = psX.tile([128, 512], F32, tag="psX", name="psX")
                for i in range(2):
                    nc.tensor.matmul(
                        ps[:],
                        x84[:, 2 * i : 2 * i + 2, jt * 128 : (jt + 1) * 128],
                        wpb[2][:, 2 * i : 2 * i + 2, :],
                        start=(i == 0),
                        stop=(i == 1),
                        perf_mode=DR,
                        skip_group_check=True,
                    )
                nc.scalar.copy(
                    out=v8_sb[jt // 2][:, (jt % 2) * 512 : (jt % 2 + 1) * 512],
                    in_=ps[:],
                )

            e_tiles = {}

            def scores(h, jt):
                """S^T [128 j, 1024 i] (bf16) + exp -> E fp8 pair slot."""
                qT, kT = qk_sb[h * 2], qk_sb[h * 2 + 1]
                ps = psM.tile([128, HW], F32, tag="psM", name="psM")
                for ic in range(2):
                    nc.tensor.matmul(
                        ps[:, IC[ic]],
                        kT[:, jt * 128 : (jt + 1) * 128],
                        qT[:, IC[ic]],
                    )
                jp, s = jt // 2, jt % 2
                if s == 0:
                    e_tiles[(h, jp)] = epool.tile([128, 2048], F8, tag="E", name="E")
                nc.scalar.activation(
                    out=e_tiles[(h, jp)][:, s * 1024 : (s + 1) * 1024],
                    in_=ps[:],
                    func=EXP,
                    scale=SCALE / (WPS * WPS),
                    bias=ebias_sb[:],
                )

            def e3(h, jp):
                return e_tiles[(h, jp)][:].rearrange("p (s n) -> p s n", s=2)

            av_ps = {}  # (h, ic) -> psX tile
            cs_ps = {}  # h -> psX tile

            def av_mm(h, jp, ic):
                if jp == 0:
                    av_ps[(h, ic)] = psX.tile([128, 512], F32, tag="psX", name="psX")
                nc.tensor.matmul(
                    av_ps[(h, ic)][:],
                    v83[jp][:, :, h * 128 : (h + 1) * 128],
                    e3(h, jp)[:, :, IC[ic]],
                    start=(jp == 0),
                    stop=(jp == 3),
                    perf_mode=DR,
                    skip_group_check=True,
                )

            def cs_mm(h, jp):
                if jp == 0:
                    cs_ps[h] = psX.tile([128, 512], F32, tag="psX", name="psX")
                for ic in range(2):
                    nc.tensor.matmul(
                        cs_ps[h][:],
                        t23[ic],
                        e3(h, jp)[:, :, IC[ic]],
                        start=(jp == 0 and ic == 0),
                        stop=(jp == 3 and ic == 1),
                        perf_mode=DR,
                        skip_group_check=True,
                    )

            o16_tiles = {}

            def o_copy(h, ic):
                o16 = o16p.tile([128, 512], BF16, tag="o16", name="o16")
                nc.vector.tensor_copy(out=o16[:], in_=av_ps[(h, ic)][:])
                o16_tiles[(h, ic)] = o16

            def recip(h):
                with nc.allow_low_precision(reason="softmax denom recip f32r"):
                    nc.vector.reciprocal(out=csr_sb[h][:], in_=cs_ps[h][0:2, :])

            def bc_norm(h, ic):
                bc = psX.tile([128, 512], F32, tag="psX", name="psX")
                nc.tensor.matmul(
                    bc[:],
                    u2_sb[:, (1 - ic) * 128 : (2 - ic) * 128],
                    csr_sb[h][:],
                )
                nc.vector.tensor_tensor(
                    out=oT8_sb[h // 2][
                        :, (h % 2) * 1024 + ic * 512 : (h % 2) * 1024 + (ic + 1) * 512
                    ],
                    in0=o16_tiles[(h, ic)][:],
                    in1=bc[:],
                    op=MUL,
                )

            d_ps = {}

            def proj_out_mm(kc, mlist):
                if kc not in d_ps:
                    d_ps[kc] = psM.tile([128, HW], F32, tag="psM", name="psM")
                ps = d_ps[kc]
                for m in mlist:
                    for ic in range(2):
                        nc.tensor.matmul(
                            ps[:, IC[ic]],
                            wo4[:, 2 * m : 2 * m + 2, kc * 128 : (kc + 1) * 128],
                            oT83[m][:, :, IC[ic]],
                            start=(m == 0),
                            stop=(m == 1),
                            perf_mode=DR,
                            skip_group_check=True,
                        )

            def finish(kc):
                ps = d_ps[kc]
                ot = outp.tile([128, HW], F32, tag="out", name="out")
                nc.vector.scalar_tensor_tensor(
                    out=ot[:],
                    in0=ps[:],
                    scalar=bres_sb[:, kc : kc + 1],
                    in1=x16_sb[:, kc * 1024 : (kc + 1) * 1024],
                    op0=ADD,
                    op1=ADD,
                )
                nc.sync.dma_start(out=outd[kc * 128 : (kc + 1) * 128, :], in_=ot[:])

            # ---- prologue ---------------------------------------------
            proj_qk(0, 0)
            proj_qk(0, 1)
            scores(0, 0)
            scores(0, 1)
            proj_qk(1, 0)
            scores(0, 2)
            scores(0, 3)
            proj_qk(1, 1)
            proj_v(0)
            proj_v(1)
            scores(0, 4)
            scores(0, 5)
            proj_v(2)
            proj_v(3)
            scores(0, 6)
            scores(0, 7)
            proj_v(4)
            proj_v(5)
            proj_v(6)
            proj_v(7)

            # ---- head loop (uniform per-iter schedule; psX 4-slot
            # rotation is arranged so nothing waits on the slow recip) ----
            for h in range(NH):
                # jp0 slot
                if h < 3:
                    scores(h + 1, 0)
                    scores(h + 1, 1)
                if h == 0:
                    proj_qk(2, 0)
                    proj_qk(2, 1)
                # jp1 slot
                if h < 3:
                    scores(h + 1, 2)
                    scores(h + 1, 3)
                if h == 0:
                    proj_qk(3, 0)
                    proj_qk(3, 1)
                else:
                    bc_norm(h - 1, 0)
                    bc_norm(h - 1, 1)
                cs_mm(h, 0)
                cs_mm(h, 1)
                av_mm(h, 0, 0)
                av_mm(h, 1, 0)
                # jp2 slot
                if h < 3:
                    scores(h + 1, 4)
                    scores(h + 1, 5)
                cs_mm(h, 2)
                av_mm(h, 2, 0)
                av_mm(h, 0, 1)
                av_mm(h, 1, 1)
                if h == 3:
                    proj_out_mm(0, [0])
                    proj_out_mm(1, [0])
                # jp3 slot
                if h < 3:
                    scores(h + 1, 6)
                    scores(h + 1, 7)
                cs_mm(h, 3)
                av_mm(h, 3, 0)
                av_mm(h, 2, 1)
                # iter end
                av_mm(h, 3, 1)
                o_copy(h, 0)
                o_copy(h, 1)
                recip(h)

            # ---- tail -------------------------------------------------
            bc_norm(3, 0)
            bc_norm(3, 1)
            proj_out_mm(0, [1])
            proj_out_mm(1, [1])
            finish(0)
            finish(1)
            proj_out_mm(2, [0, 1])
            finish(2)
            proj_out_mm(3, [0, 1])
            finish(3)

    _split_multi_waits(nc)
    return nc


_NC_CACHE = {}


def _get_nc():
    if "nc" not in _NC_CACHE:
        _NC_CACHE["nc"] = build_attention_nc()
    return _NC_CACHE["nc"]


def _host_consts():
    f8 = ml_dtypes.float8_e4m3
    # selector weights [p, (half, s, 128)]: half ic has its column ic set,
    # in both k-subtile slots, so matmul output row ic gets the colsum
    t2 = np.zeros((128, 512), dtype=f8)
    for s in range(2):
        t2[:, s * 128 + 0] = 1.0
        t2[:, 256 + s * 128 + 1] = 1.0
    u2 = np.zeros((2, 256), dtype=np.float32)
    u2[0, 128:256] = 1.0 / WPS
    u2[1, 0:128] = 1.0 / WPS
    return t2, u2


def _prep_weights(Wp, bp, Wo, bo):
    """Reorder Wp/bp columns to [Q|K|V] blocks, pre-scale by WPS, cast fp8;
    fold the v bias through the output projection on the host."""
    f8 = ml_dtypes.float8_e4m3
    Wp = np.ascontiguousarray(Wp, dtype=np.float32)
    bp = np.ascontiguousarray(bp, dtype=np.float32).reshape(-1)
    Wo = np.ascontiguousarray(Wo, dtype=np.float32)
    bo = np.ascontiguousarray(bo, dtype=np.float32).reshape(-1)
    qcols = np.concatenate([np.arange(h * 384, h * 384 + 128) for h in range(NH)])
    kcols = qcols + 128
    vcols = qcols + 256
    order = np.concatenate([qcols, kcols, vcols])
    wp8 = (Wp[:, order] * WPS).astype(f8)
    bq = (bp[qcols] * WPS).astype(np.float32).reshape(C, 1)
    bres = (bo + Wo.T @ bp[vcols]).astype(np.float32).reshape(C, 1)
    wo8 = Wo.astype(f8)
    return wp8, bq, wo8, bres


def _pmajor(a):
    """[512, N] -> [128, 4*N]: row c = cs*128 + p lands at [p, cs*N + n]."""
    n = a.shape[1]
    return np.ascontiguousarray(a.reshape(4, 128, n).transpose(1, 0, 2).reshape(128, 4 * n))


def run_sharded(x, Wp, bp, Wo, bo, **spmd_kwargs):
    """Shard over batch, run on cores 0-7, gather.  Returns ([B,C,H,W], res)."""
    f8 = ml_dtypes.float8_e4m3
    x = np.ascontiguousarray(x, dtype=np.float32).reshape(B, C, HW)
    wp8, bq, wo8, bres = _prep_weights(Wp, bp, Wo, bo)
    # block-major: [p, (block, c-subtile, 512)]
    wp8 = np.ascontiguousarray(
        wp8.reshape(4, 128, 3, 512).transpose(1, 2, 0, 3).reshape(128, 6144)
    )
    wo8 = _pmajor(wo8)
    bq = _pmajor(bq)
    bres = _pmajor(bres)
    t2, u2 = _host_consts()
    x8 = np.stack([_pmajor(x[b].astype(f8)) for b in range(B)])
    x16 = np.stack([_pmajor(x[b].astype(ml_dtypes.bfloat16)) for b in range(B)])

    nc = _get_nc()
    in_maps = []
    for b in range(B):
        in_maps.append(
            {
                "x8": x8[b],
                "x16": x16[b],
                "wp": wp8,
                "wo": wo8,
                "bq": bq,
                "bres": bres,
                "t2": t2,
                "u2": u2,
            }
        )
    res = run_bass_kernel_spmd(nc, in_maps, core_ids=list(range(B)), **spmd_kwargs)
    h = w = int(np.sqrt(HW))
    out = np.stack([res.results[b]["out"].reshape(C, h, w) for b in range(B)])
    return out, res


def kernel(x, Wp, bp, Wo, bo):
    out, _ = run_sharded(x, Wp, bp, Wo, bo)
    return out
